# revision 10
# baseline (speedup 1.0000x reference)
"""GAT residual block (nn_GATResBlock) on 8 Trainium2 NeuronCores.

Strategy
--------
- Shard destination nodes (and their incoming edges) across the 8 cores;
  each core owns a contiguous range of 6250 dst nodes.
- Host-side graph preprocessing (sanctioned by the sharding hint): sort each
  core's edges by dst block (128 dsts per block), build padded per-block edge
  lists and int16 gather-index arrays.
- Algebraic folds: a_src = x @ (W.T @ att_src-expanded) so the attention
  logits come out of the same projection matmul; segment-softmax max-trick is
  dropped (logits are bounded, softmax is shift invariant) and the softmax is
  normalized at the *node* level: agg = (sum ex*xp[src]) / (sum ex), so no
  per-edge alpha is ever materialized.
- Device per core: one replicated projection pass builds a DRAM node table
  T1[row] = [xp | a_src]; per dst-block, dma_gather fetches the rows of the
  block's source nodes, a second small gather broadcasts a_dst from a local
  table, a one-hot (edge,dst) selection matrix is built with iota/is_equal and
  a PSUM-accumulated matmul reduces weighted messages + softmax denominators
  in one pass. Epilogue divides, adds the skip projection and applies ELU.
- int16 gather indices only span 32768 rows, so the node table is gathered by
  two calls: rows [0, 32768) ("A") and [32768, ...) ("B"); the host splits
  each block's edge list accordingly.
- The per-block gather descriptor counts are EXACT (baked at build time from
  the edge data): SWDGE descriptor generation on the Pool engine is the
  bottleneck (~7.75 ns/descriptor, serialized), so no padded slot is ever
  gathered. Pad slots keep stale SBUF data; dloc=-1 makes the one-hot S zero
  them out of the reduction, and a one-time memset of the gather buffers
  keeps the first rotation NaN-free.
"""

import sys
import types

sys.path.insert(0, "/opt/trn_rl_repo")

import numpy as np
import ml_dtypes

BFDT = ml_dtypes.bfloat16


# ---------------------------------------------------------------------------
# NTFF profile hook (missing antenv.axon_hooks in this image). Needed only
# when tracing; harmless otherwise.
def _install_ntff_hook():
    if "antenv.axon_hooks" in sys.modules:
        return
    try:
        hooks = types.ModuleType("antenv.axon_hooks")
        _h = [None]
        hooks.set_axon_ntff_profile_hook = lambda h: _h.__setitem__(0, h)
        hooks.get_axon_ntff_profile_hook = lambda: _h[0]
        sys.modules["antenv.axon_hooks"] = hooks
        import antenv

        antenv.axon_hooks = hooks
        from trn_agent_boot.trn_boot import _ntff_profile_via_ctypes

        hooks.set_axon_ntff_profile_hook(
            _ntff_profile_via_ctypes("/opt/axon/libaxon_pjrt.so")
        )
    except Exception:
        pass


_install_ntff_hook()

from concourse import bacc, bass, mybir, tile  # noqa: E402
from concourse.bass_utils import run_bass_kernel_spmd  # noqa: E402

F32 = mybir.dt.float32
BF16 = mybir.dt.bfloat16
I16 = mybir.dt.int16
ALU = mybir.AluOpType
ACTF = mybir.ActivationFunctionType

P = 128
NEG_SLOPE = 0.2


class Cfg:
    def __init__(self, N=50000, IN=128, H=4, C=32, E=800000, NC=8, SPLIT=32768,
                 TA=None, TB=None):
        self.N, self.IN, self.H, self.C, self.E, self.NC = N, IN, H, C, E, NC
        self.HC = H * C
        assert self.HC == 128 and IN == 128
        assert N % NC == 0
        self.NLOC = N // NC                      # owned dst nodes per core
        self.NBLK = (self.NLOC + P - 1) // P     # dst blocks per core
        self.NLOCP = self.NBLK * P               # padded local nodes
        self.SPLIT = SPLIT                       # int16 A/B table split
        nrows = 1 + N + 1                        # PAD_A + nodes + PAD_B
        self.NR = ((nrows + P - 1) // P) * P     # node-table rows (padded)
        assert self.NR - SPLIT <= 32768
        self.PAD_B = N + 1                       # table row of the B pad
        self.ROWW = 256                  # T1 bf16 cols: xp(128)+a_src(4)+pad
        self.TA, self.TB = TA, TB                # edge tiles per block (A/B)

    @property
    def T(self):
        return self.TA + self.TB


# ---------------------------------------------------------------------------
# Host-side preprocessing: edge partitioning + gather index construction.


def _wrap_idx(arr):
    """[K*128] edge-slot array -> [128, K*8] int16 'wrapped' index layout
    (index i lives at [i % 16, i // 16], replicated across the 8 groups)."""
    k16 = arr.reshape(-1, 16).T.astype(np.int16)  # [16, K*8]
    return np.tile(k16, (8, 1))                   # [128, K*8]


def preprocess(cfg, edge_index):
    """Build per-core gather index arrays from the (2, E) edge list."""
    src = np.asarray(edge_index[0], dtype=np.int64)
    dst = np.asarray(edge_index[1], dtype=np.int64)
    core = dst // cfg.NLOC
    dstl = dst - core * cfg.NLOC
    blk = dstl // P
    srow = src + 1                                # +1: table row 0 is PAD_A
    isB = (srow >= cfg.SPLIT).astype(np.int64)

    order = np.lexsort((srow, isB, blk, core))
    core_s, blk_s, isB_s = core[order], blk[order], isB[order]
    srow_s, dstl_s = srow[order], dstl[order]

    gid = ((core_s * cfg.NBLK) + blk_s) * 2 + isB_s
    ngroups = cfg.NC * cfg.NBLK * 2
    counts = np.bincount(gid, minlength=ngroups)
    starts = np.concatenate(([0], np.cumsum(counts)[:-1]))
    rank = np.arange(len(gid)) - starts[gid]

    cA = counts.reshape(cfg.NC, cfg.NBLK, 2)[:, :, 0]
    cB = counts.reshape(cfg.NC, cfg.NBLK, 2)[:, :, 1]
    if cfg.TA is None:
        cfg.TA = max(1, int(-(-cA.max() // P)))
        cfg.TB = max(1, int(-(-cB.max() // P)))
    TA, TB, T = cfg.TA, cfg.TB, cfg.T
    assert cA.max() <= TA * P and cB.max() <= TB * P

    idxA = np.zeros((cfg.NC, cfg.NBLK, TA * P), dtype=np.int64)      # pad: row 0
    idxB = np.zeros((cfg.NC, cfg.NBLK, TB * P), dtype=np.int64)
    # pad slots are never gathered (exact num_idxs): dloc=-1 so the one-hot
    # S routes them nowhere.
    dloc = np.full((cfg.NC, cfg.NBLK, T * P), -1.0, dtype=np.float32)

    a = isB_s == 0
    idxA[core_s[a], blk_s[a], rank[a]] = srow_s[a]
    dloc[core_s[a], blk_s[a], rank[a]] = (dstl_s[a] - blk_s[a] * P)
    b = ~a
    idxB[core_s[b], blk_s[b], rank[b]] = srow_s[b] - cfg.SPLIT
    dloc[core_s[b], blk_s[b], TA * P + rank[b]] = (dstl_s[b] - blk_s[b] * P)

    # Balance SWDGE descriptor padding across cores: each core processes its
    # blocks in descending-count order, so the per-position max over cores
    # (the shared program's gather count) tracks each core's own counts.
    perm = np.argsort(-(cA + cB), axis=1, kind="stable")   # [NC, NBLK]
    cA_s = np.take_along_axis(cA, perm, axis=1)
    cB_s = np.take_along_axis(cB, perm, axis=1)

    per_core = []
    for c in range(cfg.NC):
        pc = perm[c]
        wA = np.concatenate([_wrap_idx(idxA[c, b2]) for b2 in pc], axis=1)
        wB = np.concatenate([_wrap_idx(idxB[c, b2]) for b2 in pc], axis=1)
        # dloc DRAM layout [128, NBLK*T]: [p, b*T + t] = slot (b, t, p)
        dl = dloc[c][pc].reshape(cfg.NBLK, T, P).transpose(2, 0, 1).reshape(
            P, -1)
        per_core.append(dict(idxA=np.ascontiguousarray(wA),
                             idxB=np.ascontiguousarray(wB),
                             dloc=np.ascontiguousarray(dl)))
    return per_core, np.maximum(cA_s, 1), np.maximum(cB_s, 1), perm


def make_weights(cfg, W, att_src, att_dst, bias, skip_W, skip_b):
    """Fold attention vectors into the projection weights."""
    H, C, IN = cfg.H, cfg.C, cfg.IN
    A_s = np.zeros((IN, H), dtype=np.float32)
    A_d = np.zeros((IN, H), dtype=np.float32)
    for h in range(H):
        # a_src[n,h] = sum_c xp[n,h*C+c]*att_src[h,c] = x @ (W[h*C:+C].T @ att)
        A_s[:, h] = W[h * C:(h + 1) * C, :].T @ att_src[0, h]
        A_d[:, h] = W[h * C:(h + 1) * C, :].T @ att_dst[0, h]
    Wcat = np.concatenate([W.T, A_s, A_d], axis=1).astype(BFDT)  # [IN,136]
    Wsk = np.concatenate([skip_W.T, A_d], axis=1).astype(BFDT)   # [IN,132]
    bias2 = np.tile((bias + skip_b).astype(np.float32)[None, :], (P, 1))
    return Wcat, Wsk, bias2


def make_inputs(cfg, x, edge_index, W, att_src, att_dst, bias, skip_W, skip_b):
    per_core_idx, cA, cB, perm = preprocess(cfg, edge_index)
    Wcat, Wsk, bias2 = make_weights(cfg, W, att_src, att_dst, bias, skip_W,
                                    skip_b)
    xf = np.asarray(x, dtype=np.float32)
    xT = np.zeros((cfg.IN, cfg.NR), dtype=BFDT)
    xT[:, 1:1 + cfg.N] = xf.T.astype(BFDT)
    iota = np.tile(np.arange(P, dtype=np.float32)[None, :], (P, 1))
    iotap = np.tile(np.arange(P, dtype=np.float32)[:, None], (1, P))

    in_maps = []
    for c in range(cfg.NC):
        xl = np.zeros((cfg.NLOCP, cfg.IN), dtype=np.float32)
        xl[:cfg.NLOC] = xf[c * cfg.NLOC:(c + 1) * cfg.NLOC]
        # reorder local node blocks to the core's block processing order
        xl = xl.reshape(cfg.NBLK, P, cfg.IN)[perm[c]].reshape(
            cfg.NLOCP, cfg.IN)
        xTl = np.ascontiguousarray(xl.T.astype(BFDT))
        m = dict(xT=xT, xTl=xTl, Wcat=Wcat, Wsk=Wsk,
                 bias2=bias2, iota=iota, iotap=iotap,
                 **per_core_idx[c])
        in_maps.append(m)
    return in_maps, cA, cB, perm


# ---------------------------------------------------------------------------
# Device program.


def build_program(cfg, nA, nB, debug_level=99):
    """Build the per-core SPMD Bass program.

    nA/nB: per-block EXACT gather counts (max over cores per block, so one
    SPMD program serves all cores... no -- per-core programs; see caller).
    """
    nc = bacc.Bacc(None)
    TA, TB, T = cfg.TA, cfg.TB, cfg.T
    NBLK, NR, ROWW = cfg.NBLK, cfg.NR, cfg.ROWW

    xT = nc.declare_dram_parameter("xT", [cfg.IN, NR], BF16, isOutput=False)
    xTl = nc.declare_dram_parameter("xTl", [cfg.IN, cfg.NLOCP], BF16,
                                    isOutput=False)
    Wcat = nc.declare_dram_parameter("Wcat", [cfg.IN, 136], BF16,
                                     isOutput=False)
    Wsk = nc.declare_dram_parameter("Wsk", [cfg.IN, 132], BF16, isOutput=False)
    bias2 = nc.declare_dram_parameter("bias2", [P, 128], F32, isOutput=False)
    iota = nc.declare_dram_parameter("iota", [P, P], F32, isOutput=False)
    iotap = nc.declare_dram_parameter("iotap", [P, P], F32, isOutput=False)
    idxA = nc.declare_dram_parameter("idxA", [P, NBLK * TA * 8], I16,
                                     isOutput=False)
    idxB = nc.declare_dram_parameter("idxB", [P, NBLK * TB * 8], I16,
                                     isOutput=False)
    dloc = nc.declare_dram_parameter("dloc", [P, NBLK * T], F32,
                                     isOutput=False)
    out = nc.declare_dram_parameter("out", [cfg.NLOCP, 128], F32,
                                    isOutput=True)

    T1 = nc.dram_tensor("T1", [NR, ROWW], BF16)

    with tile.TileContext(nc) as tc:
        with (
            tc.tile_pool(name="const", bufs=1) as cpool,
            tc.tile_pool(name="prol", bufs=4) as prol,
            tc.tile_pool(name="main", bufs=4) as mp,
            tc.tile_pool(name="epi", bufs=2) as ep,
        ):
            # ---- constants ----
            iota_sb = cpool.tile([P, P], F32)
            nc.sync.dma_start(out=iota_sb[:], in_=iota[:])
            iotap_sb = cpool.tile([P, P], F32)
            nc.sync.dma_start(out=iotap_sb[:], in_=iotap[:])
            ident_bf = cpool.tile([P, P], BF16)
            nc.vector.tensor_tensor(out=ident_bf[:], in0=iota_sb[:],
                                    in1=iotap_sb[:], op=ALU.is_equal)
            wcat_bf = cpool.tile([P, 136], BF16)
            nc.sync.dma_start(out=wcat_bf[:], in_=Wcat[:])
            wsk_bf = cpool.tile([P, 132], BF16)
            nc.sync.dma_start(out=wsk_bf[:], in_=Wsk[:])
            bias_sb = cpool.tile([P, 128], F32)
            nc.sync.dma_start(out=bias_sb[:], in_=bias2[:])
            idxA_sb = cpool.tile([P, NBLK * TA * 8], I16)
            nc.sync.dma_start(out=idxA_sb[:], in_=idxA[:])
            idxB_sb = cpool.tile([P, NBLK * TB * 8], I16)
            nc.sync.dma_start(out=idxB_sb[:], in_=idxB[:])
            dloc_sb = cpool.tile([P, NBLK * T], F32)
            nc.sync.dma_start(out=dloc_sb[:], in_=dloc[:])
            skip_sb = cpool.tile([P, NBLK * 128], F32)
            adst_sb = cpool.tile([P, NBLK * 4], BF16)

            # ---- phase 1: global node table T1 = [xp(bf16) | a_src] ----
            with tc.tile_pool(name="pp", bufs=2, space="PSUM") as pp:
                CH = 3
                for i0 in range(0, NR // P, CH):
                    ch = min(CH, NR // P - i0)
                    xtb = prol.tile([P, CH * P], BF16, tag="xtb")
                    nc.sync.dma_start(
                        out=xtb[:, 0:ch * P], in_=xT[:, i0 * P:(i0 + ch) * P])
                    ps = pp.tile([P, CH, 136], F32, tag="ps")
                    for k in range(ch):
                        nc.tensor.matmul(out=ps[:, k, :],
                                         lhsT=xtb[:, k * P:(k + 1) * P],
                                         rhs=wcat_bf[:], start=True, stop=True)
                    st4 = prol.tile([P, CH, 132], BF16, tag="st4")
                    nc.scalar.activation(out=st4[:, 0:ch, :],
                                         in_=ps[:, 0:ch, 0:132],
                                         func=ACTF.Copy)
                    nc.gpsimd.dma_start(
                        out=T1[i0 * P:(i0 + ch) * P, 0:132].rearrange(
                            "(k p) c -> p k c", p=P),
                        in_=st4[:, 0:ch, :])

            # ---- phase 3: per-dst-block edge processing (phase 2 bits --
            # the local skip projection / a_dst -- are interleaved into the
            # loop so they run during the gather window) ----

            # first-rotation memset: the exact-count gathers leave pad slots
            # stale; buffers must hold finite bf16 before first use.
            for _ in range(4):
                g0 = mp.tile([P, T, ROWW], BF16, tag="G1")
                nc.vector.memset(g0[:], 0.0)

            with (
                tc.tile_pool(name="acc", bufs=2, space="PSUM") as ap,
                tc.tile_pool(name="stp", bufs=2, space="PSUM") as sp,
                tc.tile_pool(name="adp", bufs=2, space="PSUM") as adp,
                tc.tile_pool(name="psk", bufs=2, space="PSUM") as ppk,
            ):
              for b in range(NBLK):
                    G1 = mp.tile([P, T, ROWW], BF16, tag="G1")
                    kA = int(nA[b])
                    kB = int(nB[b])
                    tA = (kA + P - 1) // P
                    tB = (kB + P - 1) // P
                    nc.gpsimd.dma_gather(
                        out_ap=G1[:, 0:tA, :],
                        in_ap=T1[:],
                        idxs_ap=idxA_sb[:, b * TA * 8:
                                        b * TA * 8 + ((kA + 15) // 16)],
                        num_idxs=kA,
                        num_idxs_reg=kA,
                        elem_size=ROWW,
                        single_packet=False,
                    )
                    nc.gpsimd.dma_gather(
                        out_ap=G1[:, TA:TA + tB, :],
                        in_ap=T1[cfg.SPLIT:, :],
                        idxs_ap=idxB_sb[:, b * TB * 8:
                                        b * TB * 8 + ((kB + 15) // 16)],
                        num_idxs=kB,
                        num_idxs_reg=kB,
                        elem_size=ROWW,
                        single_packet=False,
                    )

                    # interleaved phase 2: local skip projection + a_dst
                    xl = prol.tile([P, P], BF16, tag="xl")
                    nc.sync.dma_start(out=xl[:], in_=xTl[:, b * P:(b + 1) * P])
                    ps2 = ppk.tile([P, 132], F32, tag="ps2")
                    nc.tensor.matmul(out=ps2[:], lhsT=xl[:], rhs=wsk_bf[:],
                                     start=True, stop=True)
                    nc.vector.tensor_tensor(out=skip_sb[:, b * P:(b + 1) * P],
                                            in0=ps2[:, 0:128], in1=bias_sb[:],
                                            op=ALU.add)
                    nc.vector.tensor_copy(out=adst_sb[:, b * 4:(b + 1) * 4],
                                          in_=ps2[:, 128:132])

                    def gsl(t0, tn, c0, c1):
                        return G1[:, t0:t0 + tn, c0:c1]

                    # one-hot S[e, d] = (dloc[e] == d), bf16
                    S = mp.tile([P, T, P], BF16)
                    nc.vector.tensor_tensor(
                        out=S[:],
                        in0=dloc_sb[:, b * T:(b + 1) * T,
                                    None].to_broadcast([P, T, P]),
                        in1=iota_sb[:, None, :].to_broadcast([P, T, P]),
                        op=ALU.is_equal,
                    )
                    # a_dst -> per-edge via St = S^T + tiny matmul
                    elog = mp.tile([P, T, 4], F32)
                    for t0 in range(0, T, 4):
                        tn = min(4, T - t0)
                        stps = sp.tile([P, 512], BF16, tag="stps")
                        for k in range(tn):
                            nc.tensor.transpose(
                                out=stps[:, k * P:(k + 1) * P],
                                in_=S[:, t0 + k, :], identity=ident_bf[:])
                        st4b = mp.tile([P, 4, P], BF16, tag="st4b")
                        nc.vector.tensor_copy(out=st4b[:, 0:tn, :],
                                              in_=stps[:, 0:tn * P])
                        adps = adp.tile([P, 16], F32, tag="adps")
                        for k in range(tn):
                            nc.tensor.matmul(
                                out=adps[:, k * 4:(k + 1) * 4],
                                lhsT=st4b[:, k, :],
                                rhs=adst_sb[:, b * 4:(b + 1) * 4],
                                start=True, stop=True)
                        for (u0, un, p0) in (
                                [(t0, tn, 0)] if (t0 >= TA or
                                                  t0 + tn <= TA)
                                else [(t0, TA - t0, 0),
                                      (TA, t0 + tn - TA, TA - t0)]):
                            nc.vector.tensor_tensor(
                                out=elog[:, u0:u0 + un, :],
                                in0=gsl(u0, un, 128, 132),
                                in1=adps[:, p0 * 4:(p0 + un) * 4].rearrange(
                                    "p (t f) -> p t f", f=4),
                                op=ALU.add)
                    el2 = mp.tile([P, T, 4], F32)
                    nc.vector.scalar_tensor_tensor(
                        out=el2[:], in0=elog[:], scalar=NEG_SLOPE,
                        in1=elog[:], op0=ALU.mult, op1=ALU.max)
                    el3 = mp.tile([P, T, 4], F32)
                    nc.vector.tensor_scalar_max(out=el3[:], in0=el2[:],
                                                scalar1=-87.0)
                    ex = mp.tile([P, T, 4], F32)
                    nc.scalar.activation(out=ex[:], in_=el3[:],
                                         func=ACTF.Exp)
                    # V = [ex * xp | ex]  (bf16)
                    V = mp.tile([P, T, 132], BF16)
                    for (u0, un) in ((0, TA), (TA, TB)):
                        nc.vector.tensor_tensor(
                            out=V[:, u0:u0 + un, 0:128].rearrange(
                                "p t (h c) -> p t h c", c=32),
                            in0=gsl(u0, un, 0, 128).rearrange(
                                "p t (h c) -> p t h c", c=32),
                            in1=ex[:, u0:u0 + un, :,
                                   None].to_broadcast([P, un, 4, 32]),
                            op=ALU.mult,
                        )
                    nc.vector.tensor_copy(out=V[:, :, 128:132], in_=ex[:])
                    acc = ap.tile([P, 132], F32)
                    for t in range(T):
                        nc.tensor.matmul(out=acc[:], lhsT=S[:, t, :],
                                         rhs=V[:, t, :], start=(t == 0),
                                         stop=(t == T - 1))
                    # epilogue: divide, + skip, ELU
                    dn = ep.tile([P, 4], F32)
                    nc.vector.tensor_scalar_add(out=dn[:],
                                                in0=acc[:, 128:132],
                                                scalar1=1e-6)
                    rcp = ep.tile([P, 4], F32)
                    nc.vector.reciprocal(out=rcp[:], in_=dn[:])
                    y = ep.tile([P, 128], F32)
                    nc.vector.tensor_tensor(
                        out=y[:].rearrange("p (h c) -> p h c", c=32),
                        in0=acc[:, 0:128].rearrange("p (h c) -> p h c",
                                                    c=32),
                        in1=rcp[:, :, None].to_broadcast([P, 4, 32]),
                        op=ALU.mult,
                    )
                    y2 = ep.tile([P, 128], F32)
                    nc.vector.tensor_tensor(
                        out=y2[:], in0=y[:],
                        in1=skip_sb[:, b * P:(b + 1) * P], op=ALU.add)
                    # elu(v) = max(v,0) + exp(min(v,0)) - 1
                    mn = ep.tile([P, 128], F32)
                    nc.vector.tensor_scalar_min(out=mn[:], in0=y2[:],
                                                scalar1=0.0)
                    e1 = ep.tile([P, 128], F32)
                    nc.scalar.activation(out=e1[:], in_=mn[:],
                                         func=ACTF.Exp)
                    mx = ep.tile([P, 128], F32)
                    nc.vector.tensor_scalar_max(out=mx[:], in0=y2[:],
                                                scalar1=0.0)
                    yo = ep.tile([P, 128], F32)
                    nc.vector.scalar_tensor_tensor(
                        out=yo[:], in0=mx[:], scalar=-1.0, in1=e1[:],
                        op0=ALU.add, op1=ALU.add)
                    nc.scalar.dma_start(out=out[b * P:(b + 1) * P, :],
                                        in_=yo[:])

    nc.compile()
    return nc


# ---------------------------------------------------------------------------
# Public entry point.


def run_full(inputs, trace=False, **spmd_kwargs):
    cfg = Cfg()
    in_maps, cA, cB, perm = make_inputs(cfg,
                                        **{k: np.asarray(v) for k, v in
                                           inputs.items()})
    # One SPMD program must serve all 8 cores: use the per-position max count
    # across cores (tight, since each core sorted its blocks by count).
    nA = cA.max(axis=0)
    nB = cB.max(axis=0)
    nc = build_program(cfg, nA, nB)
    res = run_bass_kernel_spmd(nc, in_maps, list(range(cfg.NC)), trace=trace,
                               **spmd_kwargs)
    outs = []
    for c in range(cfg.NC):
        o = np.asarray(res.results[c]["out"]).reshape(cfg.NBLK, P, 128)
        inv = np.empty(cfg.NBLK, dtype=np.int64)
        inv[perm[c]] = np.arange(cfg.NBLK)
        outs.append(o[inv].reshape(cfg.NLOCP, 128)[:cfg.NLOC])
    return np.concatenate(outs, axis=0).astype(np.float32), res


def kernel(x, edge_index, W, att_src, att_dst, bias, skip_W, skip_b):
    out, _ = run_full(dict(x=x, edge_index=edge_index, W=W, att_src=att_src,
                           att_dst=att_dst, bias=bias, skip_W=skip_W,
                           skip_b=skip_b))
    return out


# revision 16
# speedup vs baseline: 1.0519x; 1.0519x over previous
"""GAT residual block (nn_GATResBlock) on 8 Trainium2 NeuronCores.

Strategy
--------
- Shard destination nodes (and their incoming edges) across the 8 cores;
  each core owns a contiguous range of 6250 dst nodes.
- Host-side graph preprocessing (sanctioned by the sharding hint): sort each
  core's edges by dst block (128 dsts per block), build padded per-block edge
  lists and int16 gather-index arrays.
- Algebraic folds: a_src = x @ (W.T @ att_src-expanded) so the attention
  logits come out of the same projection matmul; segment-softmax max-trick is
  dropped (logits are bounded, softmax is shift invariant) and the softmax is
  normalized at the *node* level: agg = (sum ex*xp[src]) / (sum ex), so no
  per-edge alpha is ever materialized.
- Device per core: one replicated projection pass builds a DRAM node table
  T1[row] = [xp | a_src]; per dst-block, dma_gather fetches the rows of the
  block's source nodes, a second small gather broadcasts a_dst from a local
  table, a one-hot (edge,dst) selection matrix is built with iota/is_equal and
  a PSUM-accumulated matmul reduces weighted messages + softmax denominators
  in one pass. Epilogue divides, adds the skip projection and applies ELU.
- int16 gather indices only span 32768 rows, so the node table is gathered by
  two calls: rows [0, 32768) ("A") and [32768, ...) ("B"); the host splits
  each block's edge list accordingly.
- The per-block gather descriptor counts are EXACT (baked at build time from
  the edge data): SWDGE descriptor generation on the Pool engine is the
  bottleneck (~7.75 ns/descriptor, serialized), so no padded slot is ever
  gathered. Pad slots keep stale SBUF data; dloc=-1 makes the one-hot S zero
  them out of the reduction, and a one-time memset of the gather buffers
  keeps the first rotation NaN-free.
"""

import sys
import types

sys.path.insert(0, "/opt/trn_rl_repo")

import numpy as np
import ml_dtypes

BFDT = ml_dtypes.bfloat16


# ---------------------------------------------------------------------------
# NTFF profile hook (missing antenv.axon_hooks in this image). Needed only
# when tracing; harmless otherwise.
def _install_ntff_hook():
    if "antenv.axon_hooks" in sys.modules:
        return
    try:
        hooks = types.ModuleType("antenv.axon_hooks")
        _h = [None]
        hooks.set_axon_ntff_profile_hook = lambda h: _h.__setitem__(0, h)
        hooks.get_axon_ntff_profile_hook = lambda: _h[0]
        sys.modules["antenv.axon_hooks"] = hooks
        import antenv

        antenv.axon_hooks = hooks
        from trn_agent_boot.trn_boot import _ntff_profile_via_ctypes

        hooks.set_axon_ntff_profile_hook(
            _ntff_profile_via_ctypes("/opt/axon/libaxon_pjrt.so")
        )
    except Exception:
        pass


_install_ntff_hook()

from concourse import bacc, bass, mybir, tile  # noqa: E402
from concourse.bass_utils import run_bass_kernel_spmd  # noqa: E402

F32 = mybir.dt.float32
BF16 = mybir.dt.bfloat16
I16 = mybir.dt.int16
ALU = mybir.AluOpType
ACTF = mybir.ActivationFunctionType

P = 128
NEG_SLOPE = 0.2


class Cfg:
    def __init__(self, N=50000, IN=128, H=4, C=32, E=800000, NC=8, SPLIT=32768,
                 TA=None, TB=None):
        self.N, self.IN, self.H, self.C, self.E, self.NC = N, IN, H, C, E, NC
        self.HC = H * C
        assert self.HC == 128 and IN == 128
        assert N % NC == 0
        self.NLOC = N // NC                      # owned dst nodes per core
        self.NBLK = (self.NLOC + P - 1) // P     # dst blocks per core
        self.NLOCP = self.NBLK * P               # padded local nodes
        self.SPLIT = SPLIT                       # int16 A/B table split
        nrows = 1 + N + 1                        # PAD_A + nodes + PAD_B
        self.NR = ((nrows + P - 1) // P) * P     # node-table rows (padded)
        assert self.NR - SPLIT <= 32768
        self.PAD_B = N + 1                       # table row of the B pad
        self.ROWW = 256                  # T1 bf16 cols: xp(128)+a_src(4)+pad
        self.TA, self.TB = TA, TB                # edge tiles per block (A/B)

    @property
    def T(self):
        return self.TA + self.TB


# ---------------------------------------------------------------------------
# Host-side preprocessing: edge partitioning + gather index construction.


def _wrap_idx(arr):
    """[K*128] edge-slot array -> [128, K*8] int16 'wrapped' index layout
    (index i lives at [i % 16, i // 16], replicated across the 8 groups)."""
    k16 = arr.reshape(-1, 16).T.astype(np.int16)  # [16, K*8]
    return np.tile(k16, (8, 1))                   # [128, K*8]


def preprocess(cfg, edge_index):
    """Build per-core gather index arrays from the (2, E) edge list."""
    src = np.asarray(edge_index[0], dtype=np.int64)
    dst = np.asarray(edge_index[1], dtype=np.int64)
    core = dst // cfg.NLOC
    dstl = dst - core * cfg.NLOC
    blk = dstl // P
    srow = src + 1                                # +1: table row 0 is PAD_A
    isB = (srow >= cfg.SPLIT).astype(np.int64)

    order = np.lexsort((srow, isB, blk, core))
    core_s, blk_s, isB_s = core[order], blk[order], isB[order]
    srow_s, dstl_s = srow[order], dstl[order]

    gid = ((core_s * cfg.NBLK) + blk_s) * 2 + isB_s
    ngroups = cfg.NC * cfg.NBLK * 2
    counts = np.bincount(gid, minlength=ngroups)
    starts = np.concatenate(([0], np.cumsum(counts)[:-1]))
    rank = np.arange(len(gid)) - starts[gid]

    cA = counts.reshape(cfg.NC, cfg.NBLK, 2)[:, :, 0]
    cB = counts.reshape(cfg.NC, cfg.NBLK, 2)[:, :, 1]
    if cfg.TA is None:
        cfg.TA = max(1, int(-(-cA.max() // P)))
        cfg.TB = max(1, int(-(-cB.max() // P)))
    TA, TB, T = cfg.TA, cfg.TB, cfg.T
    assert cA.max() <= TA * P and cB.max() <= TB * P

    idxA = np.zeros((cfg.NC, cfg.NBLK, TA * P), dtype=np.int64)      # pad: row 0
    idxB = np.zeros((cfg.NC, cfg.NBLK, TB * P), dtype=np.int64)
    # pad slots are never gathered (exact num_idxs): dloc=-1 so the one-hot
    # S routes them nowhere.
    dloc = np.full((cfg.NC, cfg.NBLK, T * P), -1.0, dtype=np.float32)

    a = isB_s == 0
    idxA[core_s[a], blk_s[a], rank[a]] = srow_s[a]
    dloc[core_s[a], blk_s[a], rank[a]] = (dstl_s[a] - blk_s[a] * P)
    b = ~a
    idxB[core_s[b], blk_s[b], rank[b]] = srow_s[b] - cfg.SPLIT
    dloc[core_s[b], blk_s[b], TA * P + rank[b]] = (dstl_s[b] - blk_s[b] * P)

    # Balance SWDGE descriptor padding across cores: each core processes its
    # blocks in descending-count order, so the per-position max over cores
    # (the shared program's gather count) tracks each core's own counts.
    perm = np.argsort(-(cA + cB), axis=1, kind="stable")   # [NC, NBLK]
    cA_s = np.take_along_axis(cA, perm, axis=1)
    cB_s = np.take_along_axis(cB, perm, axis=1)

    per_core = []
    for c in range(cfg.NC):
        pc = perm[c]
        wA = np.concatenate([_wrap_idx(idxA[c, b2]) for b2 in pc], axis=1)
        wB = np.concatenate([_wrap_idx(idxB[c, b2]) for b2 in pc], axis=1)
        # dloc DRAM layout [128, NBLK*T]: [p, b*T + t] = slot (b, t, p)
        dl = dloc[c][pc].reshape(cfg.NBLK, T, P).transpose(2, 0, 1).reshape(
            P, -1)
        per_core.append(dict(idxA=np.ascontiguousarray(wA),
                             idxB=np.ascontiguousarray(wB),
                             dloc=np.ascontiguousarray(dl)))
    return per_core, np.maximum(cA_s, 1), np.maximum(cB_s, 1), perm


def make_weights(cfg, W, att_src, att_dst, bias, skip_W, skip_b):
    """Fold attention vectors into the projection weights."""
    H, C, IN = cfg.H, cfg.C, cfg.IN
    A_s = np.zeros((IN, H), dtype=np.float32)
    A_d = np.zeros((IN, H), dtype=np.float32)
    for h in range(H):
        # a_src[n,h] = sum_c xp[n,h*C+c]*att_src[h,c] = x @ (W[h*C:+C].T @ att)
        A_s[:, h] = W[h * C:(h + 1) * C, :].T @ att_src[0, h]
        A_d[:, h] = W[h * C:(h + 1) * C, :].T @ att_dst[0, h]
    Wcat = np.concatenate([W.T, A_s, A_d], axis=1).astype(BFDT)  # [IN,136]
    Wsk = np.concatenate([skip_W.T, A_d], axis=1).astype(BFDT)   # [IN,132]
    bias2 = np.tile((bias + skip_b).astype(np.float32)[None, :], (P, 1))
    return Wcat, Wsk, bias2


def make_inputs(cfg, x, edge_index, W, att_src, att_dst, bias, skip_W, skip_b):
    per_core_idx, cA, cB, perm = preprocess(cfg, edge_index)
    Wcat, Wsk, bias2 = make_weights(cfg, W, att_src, att_dst, bias, skip_W,
                                    skip_b)
    xf = np.asarray(x, dtype=np.float32)
    xT = np.zeros((cfg.IN, cfg.NR), dtype=BFDT)
    xT[:, 1:1 + cfg.N] = xf.T.astype(BFDT)
    iota = np.tile(np.arange(P, dtype=np.float32)[None, :], (P, 1))
    iotap = np.tile(np.arange(P, dtype=np.float32)[:, None], (1, P))

    in_maps = []
    for c in range(cfg.NC):
        xl = np.zeros((cfg.NLOCP, cfg.IN), dtype=np.float32)
        xl[:cfg.NLOC] = xf[c * cfg.NLOC:(c + 1) * cfg.NLOC]
        # reorder local node blocks to the core's block processing order
        xl = xl.reshape(cfg.NBLK, P, cfg.IN)[perm[c]].reshape(
            cfg.NLOCP, cfg.IN)
        xTl = np.ascontiguousarray(xl.T.astype(BFDT))
        m = dict(xT=xT, xTl=xTl, Wcat=Wcat, Wsk=Wsk,
                 bias2=bias2, iota=iota, iotap=iotap,
                 **per_core_idx[c])
        in_maps.append(m)
    return in_maps, cA, cB, perm


# ---------------------------------------------------------------------------
# Device program.


def build_program(cfg, nA, nB, debug_level=99):
    """Build the per-core SPMD Bass program.

    nA/nB: per-block EXACT gather counts (max over cores per block, so one
    SPMD program serves all cores... no -- per-core programs; see caller).
    """
    nc = bacc.Bacc(None)
    TA, TB, T = cfg.TA, cfg.TB, cfg.T
    NBLK, NR, ROWW = cfg.NBLK, cfg.NR, cfg.ROWW

    xT = nc.declare_dram_parameter("xT", [cfg.IN, NR], BF16, isOutput=False)
    xTl = nc.declare_dram_parameter("xTl", [cfg.IN, cfg.NLOCP], BF16,
                                    isOutput=False)
    Wcat = nc.declare_dram_parameter("Wcat", [cfg.IN, 136], BF16,
                                     isOutput=False)
    Wsk = nc.declare_dram_parameter("Wsk", [cfg.IN, 132], BF16, isOutput=False)
    bias2 = nc.declare_dram_parameter("bias2", [P, 128], F32, isOutput=False)
    iota = nc.declare_dram_parameter("iota", [P, P], F32, isOutput=False)
    iotap = nc.declare_dram_parameter("iotap", [P, P], F32, isOutput=False)
    idxA = nc.declare_dram_parameter("idxA", [P, NBLK * TA * 8], I16,
                                     isOutput=False)
    idxB = nc.declare_dram_parameter("idxB", [P, NBLK * TB * 8], I16,
                                     isOutput=False)
    dloc = nc.declare_dram_parameter("dloc", [P, NBLK * T], F32,
                                     isOutput=False)
    out = nc.declare_dram_parameter("out", [cfg.NLOCP, 128], F32,
                                    isOutput=True)

    T1 = nc.dram_tensor("T1", [NR, ROWW], BF16)

    with tile.TileContext(nc) as tc:
        with (
            tc.tile_pool(name="const", bufs=1) as cpool,
            tc.tile_pool(name="prol", bufs=4) as prol,
            tc.tile_pool(name="main", bufs=4) as mp,
            tc.tile_pool(name="gath", bufs=6) as gp,
            tc.tile_pool(name="epi", bufs=2) as ep,
        ):
            # ---- constants ----
            iota_sb = cpool.tile([P, P], F32)
            nc.sync.dma_start(out=iota_sb[:], in_=iota[:])
            iotap_sb = cpool.tile([P, P], F32)
            nc.sync.dma_start(out=iotap_sb[:], in_=iotap[:])
            ident_bf = cpool.tile([P, P], BF16)
            nc.vector.tensor_tensor(out=ident_bf[:], in0=iota_sb[:],
                                    in1=iotap_sb[:], op=ALU.is_equal)
            wcat_bf = cpool.tile([P, 136], BF16)
            nc.sync.dma_start(out=wcat_bf[:], in_=Wcat[:])
            wsk_bf = cpool.tile([P, 132], BF16)
            nc.sync.dma_start(out=wsk_bf[:], in_=Wsk[:])
            bias_sb = cpool.tile([P, 128], F32)
            nc.sync.dma_start(out=bias_sb[:], in_=bias2[:])
            idxA_sb = cpool.tile([P, NBLK * TA * 8], I16)
            nc.sync.dma_start(out=idxA_sb[:], in_=idxA[:])
            idxB_sb = cpool.tile([P, NBLK * TB * 8], I16)
            nc.sync.dma_start(out=idxB_sb[:], in_=idxB[:])
            dloc_sb = cpool.tile([P, NBLK * T], F32)
            nc.sync.dma_start(out=dloc_sb[:], in_=dloc[:])
            skip_sb = cpool.tile([P, NBLK * 128], F32)
            adst_sb = cpool.tile([P, NBLK * 4], BF16)

            # ---- phase 1: global node table T1 = [xp(bf16) | a_src] ----
            with tc.tile_pool(name="pp", bufs=3, space="PSUM") as pp:
                CH = 6
                for i0 in range(0, NR // P, CH):
                    ch = min(CH, NR // P - i0)
                    xtb = prol.tile([P, CH * P], BF16, tag="xtb")
                    nc.sync.dma_start(
                        out=xtb[:, 0:ch * P], in_=xT[:, i0 * P:(i0 + ch) * P])
                    st4 = prol.tile([P, CH, 132], BF16, tag="st4")
                    for h0 in range(0, ch, 3):
                        hn = min(3, ch - h0)
                        ps = pp.tile([P, 3, 136], F32, tag="ps")
                        for k in range(hn):
                            nc.tensor.matmul(out=ps[:, k, :],
                                             lhsT=xtb[:, (h0 + k) * P:
                                                      (h0 + k + 1) * P],
                                             rhs=wcat_bf[:], start=True,
                                             stop=True)
                        nc.scalar.activation(out=st4[:, h0:h0 + hn, :],
                                             in_=ps[:, 0:hn, 0:132],
                                             func=ACTF.Copy)
                    nc.sync.dma_start(
                        out=T1[i0 * P:(i0 + ch) * P, 0:132].rearrange(
                            "(k p) c -> p k c", p=P),
                        in_=st4[:, 0:ch, :])

            # ---- phase 3: per-dst-block edge processing (phase 2 bits --
            # the local skip projection / a_dst -- are interleaved into the
            # loop so they run during the gather window) ----

            # first-rotation memset: the exact-count gathers leave pad slots
            # stale; buffers must hold finite bf16 before first use.
            for _ in range(6):
                g0 = gp.tile([P, T, ROWW], BF16, tag="G1")
                nc.vector.memset(g0[:], 0.0)

            with (
                tc.tile_pool(name="acc", bufs=2, space="PSUM") as ap,
                tc.tile_pool(name="stp", bufs=2, space="PSUM") as sp,
                tc.tile_pool(name="adp", bufs=2, space="PSUM") as adp,
                tc.tile_pool(name="psk", bufs=2, space="PSUM") as ppk,
            ):
              for b in range(NBLK):
                    G1 = gp.tile([P, T, ROWW], BF16, tag="G1")
                    kA = int(nA[b])
                    kB = int(nB[b])
                    tA = (kA + P - 1) // P
                    tB = (kB + P - 1) // P
                    nc.gpsimd.dma_gather(
                        out_ap=G1[:, 0:tA, :],
                        in_ap=T1[:],
                        idxs_ap=idxA_sb[:, b * TA * 8:
                                        b * TA * 8 + ((kA + 15) // 16)],
                        num_idxs=kA,
                        num_idxs_reg=kA,
                        elem_size=ROWW,
                        single_packet=False,
                    )
                    nc.gpsimd.dma_gather(
                        out_ap=G1[:, TA:TA + tB, :],
                        in_ap=T1[cfg.SPLIT:, :],
                        idxs_ap=idxB_sb[:, b * TB * 8:
                                        b * TB * 8 + ((kB + 15) // 16)],
                        num_idxs=kB,
                        num_idxs_reg=kB,
                        elem_size=ROWW,
                        single_packet=False,
                    )

                    # interleaved phase 2: local skip projection + a_dst
                    xl = prol.tile([P, P], BF16, tag="xl")
                    nc.sync.dma_start(out=xl[:], in_=xTl[:, b * P:(b + 1) * P])
                    ps2 = ppk.tile([P, 132], F32, tag="ps2")
                    nc.tensor.matmul(out=ps2[:], lhsT=xl[:], rhs=wsk_bf[:],
                                     start=True, stop=True)
                    nc.vector.tensor_tensor(out=skip_sb[:, b * P:(b + 1) * P],
                                            in0=ps2[:, 0:128], in1=bias_sb[:],
                                            op=ALU.add)
                    nc.vector.tensor_copy(out=adst_sb[:, b * 4:(b + 1) * 4],
                                          in_=ps2[:, 128:132])

                    def gsl(t0, tn, c0, c1):
                        return G1[:, t0:t0 + tn, c0:c1]

                    # one-hot S[e, d] = (dloc[e] == d), bf16
                    S = mp.tile([P, T, P], BF16)
                    nc.vector.tensor_tensor(
                        out=S[:],
                        in0=dloc_sb[:, b * T:(b + 1) * T,
                                    None].to_broadcast([P, T, P]),
                        in1=iota_sb[:, None, :].to_broadcast([P, T, P]),
                        op=ALU.is_equal,
                    )
                    # a_dst -> per-edge via St = S^T + tiny matmul
                    elog = mp.tile([P, T, 4], F32)
                    for t0 in range(0, T, 4):
                        tn = min(4, T - t0)
                        stps = sp.tile([P, 512], BF16, tag="stps")
                        for k in range(tn):
                            nc.tensor.transpose(
                                out=stps[:, k * P:(k + 1) * P],
                                in_=S[:, t0 + k, :], identity=ident_bf[:])
                        st4b = mp.tile([P, 4, P], BF16, tag="st4b")
                        nc.vector.tensor_copy(out=st4b[:, 0:tn, :],
                                              in_=stps[:, 0:tn * P])
                        adps = adp.tile([P, 16], F32, tag="adps")
                        for k in range(tn):
                            nc.tensor.matmul(
                                out=adps[:, k * 4:(k + 1) * 4],
                                lhsT=st4b[:, k, :],
                                rhs=adst_sb[:, b * 4:(b + 1) * 4],
                                start=True, stop=True)
                        for (u0, un, p0) in (
                                [(t0, tn, 0)] if (t0 >= TA or
                                                  t0 + tn <= TA)
                                else [(t0, TA - t0, 0),
                                      (TA, t0 + tn - TA, TA - t0)]):
                            nc.vector.tensor_tensor(
                                out=elog[:, u0:u0 + un, :],
                                in0=gsl(u0, un, 128, 132),
                                in1=adps[:, p0 * 4:(p0 + un) * 4].rearrange(
                                    "p (t f) -> p t f", f=4),
                                op=ALU.add)
                    el2 = mp.tile([P, T, 4], F32)
                    nc.vector.scalar_tensor_tensor(
                        out=el2[:], in0=elog[:], scalar=NEG_SLOPE,
                        in1=elog[:], op0=ALU.mult, op1=ALU.max)
                    el3 = mp.tile([P, T, 4], F32)
                    nc.vector.tensor_scalar_max(out=el3[:], in0=el2[:],
                                                scalar1=-87.0)
                    ex = mp.tile([P, T, 4], F32)
                    nc.scalar.activation(out=ex[:], in_=el3[:],
                                         func=ACTF.Exp)
                    # V = [ex * xp | ex]  (bf16)
                    V = mp.tile([P, T, 132], BF16)
                    for (u0, un) in ((0, TA), (TA, TB)):
                        nc.vector.tensor_tensor(
                            out=V[:, u0:u0 + un, 0:128].rearrange(
                                "p t (h c) -> p t h c", c=32),
                            in0=gsl(u0, un, 0, 128).rearrange(
                                "p t (h c) -> p t h c", c=32),
                            in1=ex[:, u0:u0 + un, :,
                                   None].to_broadcast([P, un, 4, 32]),
                            op=ALU.mult,
                        )
                    nc.vector.tensor_copy(out=V[:, :, 128:132], in_=ex[:])
                    acc = ap.tile([P, 132], F32)
                    for t in range(T):
                        nc.tensor.matmul(out=acc[:], lhsT=S[:, t, :],
                                         rhs=V[:, t, :], start=(t == 0),
                                         stop=(t == T - 1))
                    # epilogue: divide, + skip, ELU
                    dn = ep.tile([P, 4], F32)
                    nc.vector.tensor_scalar_add(out=dn[:],
                                                in0=acc[:, 128:132],
                                                scalar1=1e-6)
                    rcp = ep.tile([P, 4], F32)
                    nc.vector.reciprocal(out=rcp[:], in_=dn[:])
                    y = ep.tile([P, 128], F32)
                    nc.vector.tensor_tensor(
                        out=y[:].rearrange("p (h c) -> p h c", c=32),
                        in0=acc[:, 0:128].rearrange("p (h c) -> p h c",
                                                    c=32),
                        in1=rcp[:, :, None].to_broadcast([P, 4, 32]),
                        op=ALU.mult,
                    )
                    y2 = ep.tile([P, 128], F32)
                    nc.vector.tensor_tensor(
                        out=y2[:], in0=y[:],
                        in1=skip_sb[:, b * P:(b + 1) * P], op=ALU.add)
                    # elu(v) = max(v,0) + exp(min(v,0)) - 1
                    mn = ep.tile([P, 128], F32)
                    nc.vector.tensor_scalar_min(out=mn[:], in0=y2[:],
                                                scalar1=0.0)
                    e1 = ep.tile([P, 128], F32)
                    nc.scalar.activation(out=e1[:], in_=mn[:],
                                         func=ACTF.Exp)
                    mx = ep.tile([P, 128], F32)
                    nc.vector.tensor_scalar_max(out=mx[:], in0=y2[:],
                                                scalar1=0.0)
                    yo = ep.tile([P, 128], F32)
                    nc.vector.scalar_tensor_tensor(
                        out=yo[:], in0=mx[:], scalar=-1.0, in1=e1[:],
                        op0=ALU.add, op1=ALU.add)
                    nc.scalar.dma_start(out=out[b * P:(b + 1) * P, :],
                                        in_=yo[:])

    nc.compile()
    return nc


# ---------------------------------------------------------------------------
# Public entry point.


def run_full(inputs, trace=False, **spmd_kwargs):
    cfg = Cfg()
    in_maps, cA, cB, perm = make_inputs(cfg,
                                        **{k: np.asarray(v) for k, v in
                                           inputs.items()})
    # One SPMD program must serve all 8 cores: use the per-position max count
    # across cores (tight, since each core sorted its blocks by count).
    nA = cA.max(axis=0)
    nB = cB.max(axis=0)
    nc = build_program(cfg, nA, nB)
    res = run_bass_kernel_spmd(nc, in_maps, list(range(cfg.NC)), trace=trace,
                               **spmd_kwargs)
    outs = []
    for c in range(cfg.NC):
        o = np.asarray(res.results[c]["out"]).reshape(cfg.NBLK, P, 128)
        inv = np.empty(cfg.NBLK, dtype=np.int64)
        inv[perm[c]] = np.arange(cfg.NBLK)
        outs.append(o[inv].reshape(cfg.NLOCP, 128)[:cfg.NLOC])
    return np.concatenate(outs, axis=0).astype(np.float32), res


def kernel(x, edge_index, W, att_src, att_dst, bias, skip_W, skip_b):
    out, _ = run_full(dict(x=x, edge_index=edge_index, W=W, att_src=att_src,
                           att_dst=att_dst, bias=bias, skip_W=skip_W,
                           skip_b=skip_b))
    return out


# revision 23
# speedup vs baseline: 1.0895x; 1.0358x over previous
"""GAT residual block (nn_GATResBlock) on 8 Trainium2 NeuronCores.

Strategy
--------
- Shard destination nodes (and their incoming edges) across the 8 cores;
  each core owns a contiguous range of 6250 dst nodes.
- Host-side graph preprocessing (sanctioned by the sharding hint): sort each
  core's edges by dst block (128 dsts per block), build padded per-block edge
  lists and int16 gather-index arrays.
- Algebraic folds: a_src = x @ (W.T @ att_src-expanded) so the attention
  logits come out of the same projection matmul; segment-softmax max-trick is
  dropped (logits are bounded, softmax is shift invariant) and the softmax is
  normalized at the *node* level: agg = (sum ex*xp[src]) / (sum ex), so no
  per-edge alpha is ever materialized.
- Device per core: one replicated projection pass builds a DRAM node table
  T1[row] = [xp | a_src]; per dst-block, dma_gather fetches the rows of the
  block's source nodes, a second small gather broadcasts a_dst from a local
  table, a one-hot (edge,dst) selection matrix is built with iota/is_equal and
  a PSUM-accumulated matmul reduces weighted messages + softmax denominators
  in one pass. Epilogue divides, adds the skip projection and applies ELU.
- int16 gather indices only span 32768 rows, so the node table is gathered by
  two calls: rows [0, 32768) ("A") and [32768, ...) ("B"); the host splits
  each block's edge list accordingly.
- The per-block gather descriptor counts are EXACT (baked at build time from
  the edge data): SWDGE descriptor generation on the Pool engine is the
  bottleneck (~7.75 ns/descriptor, serialized), so no padded slot is ever
  gathered. Pad slots keep stale SBUF data; dloc=-1 makes the one-hot S zero
  them out of the reduction, and a one-time memset of the gather buffers
  keeps the first rotation NaN-free.
"""

import sys
import types

sys.path.insert(0, "/opt/trn_rl_repo")

import numpy as np
import ml_dtypes

BFDT = ml_dtypes.bfloat16


# ---------------------------------------------------------------------------
# NTFF profile hook (missing antenv.axon_hooks in this image). Needed only
# when tracing; harmless otherwise.
def _install_ntff_hook():
    if "antenv.axon_hooks" in sys.modules:
        return
    try:
        hooks = types.ModuleType("antenv.axon_hooks")
        _h = [None]
        hooks.set_axon_ntff_profile_hook = lambda h: _h.__setitem__(0, h)
        hooks.get_axon_ntff_profile_hook = lambda: _h[0]
        sys.modules["antenv.axon_hooks"] = hooks
        import antenv

        antenv.axon_hooks = hooks
        from trn_agent_boot.trn_boot import _ntff_profile_via_ctypes

        hooks.set_axon_ntff_profile_hook(
            _ntff_profile_via_ctypes("/opt/axon/libaxon_pjrt.so")
        )
    except Exception:
        pass


_install_ntff_hook()

from concourse import bacc, bass, mybir, tile  # noqa: E402
from concourse.bass_utils import run_bass_kernel_spmd  # noqa: E402

F32 = mybir.dt.float32
BF16 = mybir.dt.bfloat16
I16 = mybir.dt.int16
ALU = mybir.AluOpType
ACTF = mybir.ActivationFunctionType

P = 128
NEG_SLOPE = 0.2


class Cfg:
    def __init__(self, N=50000, IN=128, H=4, C=32, E=800000, NC=8, SPLIT=32768,
                 TA=None, TB=None):
        self.N, self.IN, self.H, self.C, self.E, self.NC = N, IN, H, C, E, NC
        self.HC = H * C
        assert self.HC == 128 and IN == 128
        assert N % NC == 0
        self.NLOC = N // NC                      # owned dst nodes per core
        self.NBLK = (self.NLOC + P - 1) // P     # dst blocks per core
        self.NLOCP = self.NBLK * P               # padded local nodes
        self.SPLIT = SPLIT                       # int16 A/B table split
        nrows = 1 + N + 1                        # PAD_A + nodes + PAD_B
        self.NR = ((nrows + P - 1) // P) * P     # node-table rows (padded)
        assert self.NR - SPLIT <= 32768
        self.PAD_B = N + 1                       # table row of the B pad
        self.ROWW = 256                  # T1 bf16 cols: xp(128)+a_src(4)+pad
        self.TA, self.TB = TA, TB                # edge tiles per block (A/B)

    @property
    def T(self):
        return self.TA + self.TB


# ---------------------------------------------------------------------------
# Host-side preprocessing: edge partitioning + gather index construction.


def _wrap_idx(arr):
    """[K*128] edge-slot array -> [128, K*8] int16 'wrapped' index layout
    (index i lives at [i % 16, i // 16], replicated across the 8 groups)."""
    k16 = arr.reshape(-1, 16).T.astype(np.int16)  # [16, K*8]
    return np.tile(k16, (8, 1))                   # [128, K*8]


def preprocess(cfg, edge_index):
    """Build per-core gather index arrays from the (2, E) edge list."""
    src = np.asarray(edge_index[0], dtype=np.int64)
    dst = np.asarray(edge_index[1], dtype=np.int64)
    core = dst // cfg.NLOC
    dstl = dst - core * cfg.NLOC
    blk = dstl // P
    srow = src + 1                                # +1: table row 0 is PAD_A
    isB = (srow >= cfg.SPLIT).astype(np.int64)

    order = np.lexsort((srow, isB, blk, core))
    core_s, blk_s, isB_s = core[order], blk[order], isB[order]
    srow_s, dstl_s = srow[order], dstl[order]

    gid = ((core_s * cfg.NBLK) + blk_s) * 2 + isB_s
    ngroups = cfg.NC * cfg.NBLK * 2
    counts = np.bincount(gid, minlength=ngroups)
    starts = np.concatenate(([0], np.cumsum(counts)[:-1]))
    rank = np.arange(len(gid)) - starts[gid]

    cA = counts.reshape(cfg.NC, cfg.NBLK, 2)[:, :, 0]
    cB = counts.reshape(cfg.NC, cfg.NBLK, 2)[:, :, 1]
    if cfg.TA is None:
        cfg.TA = max(1, int(-(-cA.max() // P)))
        cfg.TB = max(1, int(-(-cB.max() // P)))
    TA, TB, T = cfg.TA, cfg.TB, cfg.T
    assert cA.max() <= TA * P and cB.max() <= TB * P

    idxA = np.zeros((cfg.NC, cfg.NBLK, TA * P), dtype=np.int64)      # pad: row 0
    idxB = np.zeros((cfg.NC, cfg.NBLK, TB * P), dtype=np.int64)
    # pad slots are never gathered (exact num_idxs): dloc=-1 so the one-hot
    # S routes them nowhere.
    dloc = np.full((cfg.NC, cfg.NBLK, T * P), -1.0, dtype=np.float32)

    a = isB_s == 0
    idxA[core_s[a], blk_s[a], rank[a]] = srow_s[a]
    dloc[core_s[a], blk_s[a], rank[a]] = (dstl_s[a] - blk_s[a] * P)
    b = ~a
    idxB[core_s[b], blk_s[b], rank[b]] = srow_s[b] - cfg.SPLIT
    dloc[core_s[b], blk_s[b], TA * P + rank[b]] = (dstl_s[b] - blk_s[b] * P)

    # Balance SWDGE descriptor padding across cores: each core processes its
    # blocks in descending-count order, so the per-position max over cores
    # (the shared program's gather count) tracks each core's own counts.
    perm = np.argsort(-(cA + cB), axis=1, kind="stable")   # [NC, NBLK]
    cA_s = np.take_along_axis(cA, perm, axis=1)
    cB_s = np.take_along_axis(cB, perm, axis=1)

    per_core = []
    for c in range(cfg.NC):
        pc = perm[c]
        wA = np.concatenate([_wrap_idx(idxA[c, b2]) for b2 in pc], axis=1)
        wB = np.concatenate([_wrap_idx(idxB[c, b2]) for b2 in pc], axis=1)
        # dloc DRAM layout [128, NBLK*T]: [p, b*T + t] = slot (b, t, p)
        dl = dloc[c][pc].reshape(cfg.NBLK, T, P).transpose(2, 0, 1).reshape(
            P, -1)
        # row layout for the S^T build: [1, b*T*P + t*P + e] (bf16-exact)
        dlT = dloc[c][pc].reshape(1, -1).astype(BFDT)
        per_core.append(dict(idxA=np.ascontiguousarray(wA),
                             idxB=np.ascontiguousarray(wB),
                             dloc=np.ascontiguousarray(dl),
                             dlocT=np.ascontiguousarray(dlT)))
    return per_core, np.maximum(cA_s, 1), np.maximum(cB_s, 1), perm


def make_weights(cfg, W, att_src, att_dst, bias, skip_W, skip_b):
    """Fold attention vectors into the projection weights."""
    H, C, IN = cfg.H, cfg.C, cfg.IN
    A_s = np.zeros((IN, H), dtype=np.float32)
    A_d = np.zeros((IN, H), dtype=np.float32)
    for h in range(H):
        # a_src[n,h] = sum_c xp[n,h*C+c]*att_src[h,c] = x @ (W[h*C:+C].T @ att)
        A_s[:, h] = W[h * C:(h + 1) * C, :].T @ att_src[0, h]
        A_d[:, h] = W[h * C:(h + 1) * C, :].T @ att_dst[0, h]
    Wcat = np.concatenate([W.T, A_s, A_d], axis=1).astype(BFDT)  # [IN,136]
    Wsk = np.concatenate([skip_W.T, A_d], axis=1).astype(BFDT)   # [IN,132]
    bias2 = np.tile((bias + skip_b).astype(np.float32)[None, :], (P, 1))
    return Wcat, Wsk, bias2


def make_inputs(cfg, x, edge_index, W, att_src, att_dst, bias, skip_W, skip_b):
    per_core_idx, cA, cB, perm = preprocess(cfg, edge_index)
    Wcat, Wsk, bias2 = make_weights(cfg, W, att_src, att_dst, bias, skip_W,
                                    skip_b)
    xf = np.asarray(x, dtype=np.float32)
    xT = np.zeros((cfg.IN, cfg.NR), dtype=BFDT)
    xT[:, 1:1 + cfg.N] = xf.T.astype(BFDT)
    iota = np.tile(np.arange(P, dtype=np.float32)[None, :], (P, 1))
    iotap = np.tile(np.arange(P, dtype=np.float32)[:, None], (1, P))

    in_maps = []
    for c in range(cfg.NC):
        xl = np.zeros((cfg.NLOCP, cfg.IN), dtype=np.float32)
        xl[:cfg.NLOC] = xf[c * cfg.NLOC:(c + 1) * cfg.NLOC]
        # reorder local node blocks to the core's block processing order
        xl = xl.reshape(cfg.NBLK, P, cfg.IN)[perm[c]].reshape(
            cfg.NLOCP, cfg.IN)
        xTl = np.ascontiguousarray(xl.T.astype(BFDT))
        m = dict(xT=xT, xTl=xTl, Wcat=Wcat, Wsk=Wsk,
                 bias2=bias2, iota=iota, iotap=iotap,
                 ones1=np.ones((1, P), dtype=BFDT),
                 **per_core_idx[c])
        in_maps.append(m)
    return in_maps, cA, cB, perm


# ---------------------------------------------------------------------------
# Device program.


def build_program(cfg, nA, nB, debug_level=99):
    """Build the per-core SPMD Bass program.

    nA/nB: per-block EXACT gather counts (max over cores per block, so one
    SPMD program serves all cores... no -- per-core programs; see caller).
    """
    nc = bacc.Bacc(None)
    TA, TB, T = cfg.TA, cfg.TB, cfg.T
    NBLK, NR, ROWW = cfg.NBLK, cfg.NR, cfg.ROWW

    xT = nc.declare_dram_parameter("xT", [cfg.IN, NR], BF16, isOutput=False)
    xTl = nc.declare_dram_parameter("xTl", [cfg.IN, cfg.NLOCP], BF16,
                                    isOutput=False)
    Wcat = nc.declare_dram_parameter("Wcat", [cfg.IN, 136], BF16,
                                     isOutput=False)
    Wsk = nc.declare_dram_parameter("Wsk", [cfg.IN, 132], BF16, isOutput=False)
    bias2 = nc.declare_dram_parameter("bias2", [P, 128], F32, isOutput=False)
    iota = nc.declare_dram_parameter("iota", [P, P], F32, isOutput=False)
    iotap = nc.declare_dram_parameter("iotap", [P, P], F32, isOutput=False)
    idxA = nc.declare_dram_parameter("idxA", [P, NBLK * TA * 8], I16,
                                     isOutput=False)
    idxB = nc.declare_dram_parameter("idxB", [P, NBLK * TB * 8], I16,
                                     isOutput=False)
    dloc = nc.declare_dram_parameter("dloc", [P, NBLK * T], F32,
                                     isOutput=False)
    dlocT = nc.declare_dram_parameter("dlocT", [1, NBLK * T * P], BF16,
                                      isOutput=False)
    ones1 = nc.declare_dram_parameter("ones1", [1, P], BF16, isOutput=False)
    out = nc.declare_dram_parameter("out", [cfg.NLOCP, 128], F32,
                                    isOutput=True)

    T1 = nc.dram_tensor("T1", [NR, ROWW], BF16)

    with tile.TileContext(nc) as tc:
        with (
            tc.tile_pool(name="const", bufs=1) as cpool,
            tc.tile_pool(name="prol", bufs=4) as prol,
            tc.tile_pool(name="main", bufs=4) as mp,
            tc.tile_pool(name="gath", bufs=6) as gp,
            tc.tile_pool(name="epi", bufs=2) as ep,
        ):
            # ---- constants ----
            iota_sb = cpool.tile([P, P], F32)
            nc.sync.dma_start(out=iota_sb[:], in_=iota[:])
            iotap_sb = cpool.tile([P, P], F32)
            nc.sync.dma_start(out=iotap_sb[:], in_=iotap[:])
            ones_sb = cpool.tile([1, P], BF16)
            nc.sync.dma_start(out=ones_sb[:], in_=ones1[:])
            wcat_bf = cpool.tile([P, 136], BF16)
            nc.sync.dma_start(out=wcat_bf[:], in_=Wcat[:])
            wsk_bf = cpool.tile([P, 132], BF16)
            nc.sync.dma_start(out=wsk_bf[:], in_=Wsk[:])
            bias_sb = cpool.tile([P, 128], F32)
            nc.sync.dma_start(out=bias_sb[:], in_=bias2[:])
            idxA_sb = cpool.tile([P, NBLK * TA * 8], I16)
            nc.sync.dma_start(out=idxA_sb[:], in_=idxA[:])
            idxB_sb = cpool.tile([P, NBLK * TB * 8], I16)
            nc.sync.dma_start(out=idxB_sb[:], in_=idxB[:])
            dloc_sb = cpool.tile([P, NBLK * T], F32)
            nc.sync.dma_start(out=dloc_sb[:], in_=dloc[:])
            skip_sb = cpool.tile([P, NBLK * 128], F32)
            adst_sb = cpool.tile([P, NBLK * 4], BF16)

            # ---- phase 1: global node table T1 = [xp(bf16) | a_src] ----
            with tc.tile_pool(name="pp", bufs=3, space="PSUM") as pp:
                CH = 6
                for i0 in range(0, NR // P, CH):
                    ch = min(CH, NR // P - i0)
                    xtb = prol.tile([P, CH * P], BF16, tag="xtb")
                    nc.gpsimd.dma_start(
                        out=xtb[:, 0:ch * P], in_=xT[:, i0 * P:(i0 + ch) * P])
                    st4 = prol.tile([P, CH, 132], BF16, tag="st4")
                    for h0 in range(0, ch, 3):
                        hn = min(3, ch - h0)
                        ps = pp.tile([P, 3, 136], F32, tag="ps")
                        for k in range(hn):
                            nc.tensor.matmul(out=ps[:, k, :],
                                             lhsT=xtb[:, (h0 + k) * P:
                                                      (h0 + k + 1) * P],
                                             rhs=wcat_bf[:], start=True,
                                             stop=True)
                        if h0 == 0:
                            nc.scalar.activation(out=st4[:, h0:h0 + hn, :],
                                                 in_=ps[:, 0:hn, 0:132],
                                                 func=ACTF.Copy)
                        else:
                            nc.vector.tensor_copy(out=st4[:, h0:h0 + hn, :],
                                                  in_=ps[:, 0:hn, 0:132])
                    nc.sync.dma_start(
                        out=T1[i0 * P:(i0 + ch) * P, 0:132].rearrange(
                            "(k p) c -> p k c", p=P),
                        in_=st4[:, 0:ch, :])

            # ---- phase 3: per-dst-block edge processing (phase 2 bits --
            # the local skip projection / a_dst -- are interleaved into the
            # loop so they run during the gather window) ----

            # first-rotation memset: the exact-count gathers leave pad slots
            # stale; buffers must hold finite bf16 before first use.
            for _ in range(6):
                g0 = gp.tile([P, T, ROWW], BF16, tag="G1")
                nc.vector.memset(g0[:], 0.0)

            with (
                tc.tile_pool(name="acc", bufs=2, space="PSUM") as ap,
                tc.tile_pool(name="stp", bufs=2, space="PSUM") as sp,
                tc.tile_pool(name="adp", bufs=2, space="PSUM") as adp,
                tc.tile_pool(name="psk", bufs=2, space="PSUM") as ppk,
            ):
              for b in range(NBLK):
                    G1 = gp.tile([P, T, ROWW], BF16, tag="G1")
                    kA = int(nA[b])
                    kB = int(nB[b])
                    tA = (kA + P - 1) // P
                    tB = (kB + P - 1) // P
                    nc.gpsimd.dma_gather(
                        out_ap=G1[:, 0:tA, :],
                        in_ap=T1[:],
                        idxs_ap=idxA_sb[:, b * TA * 8:
                                        b * TA * 8 + ((kA + 15) // 16)],
                        num_idxs=kA,
                        num_idxs_reg=kA,
                        elem_size=ROWW,
                        single_packet=False,
                    )
                    nc.gpsimd.dma_gather(
                        out_ap=G1[:, TA:TA + tB, :],
                        in_ap=T1[cfg.SPLIT:, :],
                        idxs_ap=idxB_sb[:, b * TB * 8:
                                        b * TB * 8 + ((kB + 15) // 16)],
                        num_idxs=kB,
                        num_idxs_reg=kB,
                        elem_size=ROWW,
                        single_packet=False,
                    )

                    # interleaved phase 2: local skip projection + a_dst
                    xl = prol.tile([P, P], BF16, tag="xl")
                    nc.sync.dma_start(out=xl[:], in_=xTl[:, b * P:(b + 1) * P])
                    ps2 = ppk.tile([P, 132], F32, tag="ps2")
                    nc.tensor.matmul(out=ps2[:], lhsT=xl[:], rhs=wsk_bf[:],
                                     start=True, stop=True)
                    nc.vector.tensor_tensor(out=skip_sb[:, b * P:(b + 1) * P],
                                            in0=ps2[:, 0:128], in1=bias_sb[:],
                                            op=ALU.add)
                    nc.vector.tensor_copy(out=adst_sb[:, b * 4:(b + 1) * 4],
                                          in_=ps2[:, 128:132])

                    def gsl(t0, tn, c0, c1):
                        return G1[:, t0:t0 + tn, c0:c1]

                    # one-hot S[e, d] = (dloc[e] == d), bf16
                    S = mp.tile([P, T, P], BF16)
                    nc.vector.tensor_tensor(
                        out=S[:],
                        in0=dloc_sb[:, b * T:(b + 1) * T,
                                    None].to_broadcast([P, T, P]),
                        in1=iota_sb[:, None, :].to_broadcast([P, T, P]),
                        op=ALU.is_equal,
                    )
                    # S^T built directly: broadcast dlocT across partitions
                    # via a ones-vector matmul, then is_equal with the
                    # partition index.
                    dlT_sb = prol.tile([1, T * P], BF16, tag="dlT")
                    nc.scalar.dma_start(
                        out=dlT_sb[:],
                        in_=dlocT[0:1, b * T * P:(b + 1) * T * P])
                    St = mp.tile([P, T, P], BF16, tag="St")
                    for t0 in range(0, T, 4):
                        tn = min(4, T - t0)
                        dbc = sp.tile([P, 512], F32, tag="dbc")
                        nc.tensor.matmul(
                            out=dbc[:, 0:tn * P],
                            lhsT=ones_sb[:],
                            rhs=dlT_sb[0:1, t0 * P:(t0 + tn) * P],
                            start=True, stop=True)
                        nc.vector.tensor_tensor(
                            out=St[:, t0:t0 + tn, :],
                            in0=iotap_sb[:, 0:1, None].to_broadcast(
                                [P, tn, P]),
                            in1=dbc[:, 0:tn * P].rearrange(
                                "p (t e) -> p t e", e=P),
                            op=ALU.is_equal,
                        )
                    # a_dst -> per-edge via tiny matmuls off S^T
                    elog = mp.tile([P, T, 4], F32)
                    for t0 in range(0, T, 4):
                        tn = min(4, T - t0)
                        adps = adp.tile([P, 16], F32, tag="adps")
                        for k in range(tn):
                            nc.tensor.matmul(
                                out=adps[:, k * 4:(k + 1) * 4],
                                lhsT=St[:, t0 + k, :],
                                rhs=adst_sb[:, b * 4:(b + 1) * 4],
                                start=True, stop=True)
                        for (u0, un, p0) in (
                                [(t0, tn, 0)] if (t0 >= TA or
                                                  t0 + tn <= TA)
                                else [(t0, TA - t0, 0),
                                      (TA, t0 + tn - TA, TA - t0)]):
                            nc.vector.tensor_tensor(
                                out=elog[:, u0:u0 + un, :],
                                in0=gsl(u0, un, 128, 132),
                                in1=adps[:, p0 * 4:(p0 + un) * 4].rearrange(
                                    "p (t f) -> p t f", f=4),
                                op=ALU.add)
                    el2 = mp.tile([P, T, 4], F32)
                    nc.vector.scalar_tensor_tensor(
                        out=el2[:], in0=elog[:], scalar=NEG_SLOPE,
                        in1=elog[:], op0=ALU.mult, op1=ALU.max)
                    ex = mp.tile([P, T, 4], F32)
                    nc.scalar.activation(out=ex[:], in_=el2[:],
                                         func=ACTF.Exp)
                    # V = [ex * xp | ex]  (bf16)
                    V = mp.tile([P, T, 132], BF16)
                    for (u0, un) in ((0, TA), (TA, TB)):
                        nc.vector.tensor_tensor(
                            out=V[:, u0:u0 + un, 0:128].rearrange(
                                "p t (h c) -> p t h c", c=32),
                            in0=gsl(u0, un, 0, 128).rearrange(
                                "p t (h c) -> p t h c", c=32),
                            in1=ex[:, u0:u0 + un, :,
                                   None].to_broadcast([P, un, 4, 32]),
                            op=ALU.mult,
                        )
                    nc.vector.tensor_copy(out=V[:, :, 128:132], in_=ex[:])
                    acc = ap.tile([P, 132], F32)
                    for t in range(T):
                        nc.tensor.matmul(out=acc[:], lhsT=S[:, t, :],
                                         rhs=V[:, t, :], start=(t == 0),
                                         stop=(t == T - 1))
                    # epilogue: divide, + skip, ELU
                    dn = ep.tile([P, 4], F32)
                    nc.vector.tensor_scalar_add(out=dn[:],
                                                in0=acc[:, 128:132],
                                                scalar1=1e-6)
                    rcp = ep.tile([P, 4], F32)
                    nc.vector.reciprocal(out=rcp[:], in_=dn[:])
                    y = ep.tile([P, 128], F32)
                    nc.vector.tensor_tensor(
                        out=y[:].rearrange("p (h c) -> p h c", c=32),
                        in0=acc[:, 0:128].rearrange("p (h c) -> p h c",
                                                    c=32),
                        in1=rcp[:, :, None].to_broadcast([P, 4, 32]),
                        op=ALU.mult,
                    )
                    y2 = ep.tile([P, 128], F32)
                    nc.vector.tensor_tensor(
                        out=y2[:], in0=y[:],
                        in1=skip_sb[:, b * P:(b + 1) * P], op=ALU.add)
                    # elu(v) = max(v,0) + exp(min(v,0)) - 1
                    mn = ep.tile([P, 128], F32)
                    nc.vector.tensor_scalar_min(out=mn[:], in0=y2[:],
                                                scalar1=0.0)
                    e1 = ep.tile([P, 128], F32)
                    nc.scalar.activation(out=e1[:], in_=mn[:],
                                         func=ACTF.Exp)
                    mx = ep.tile([P, 128], F32)
                    nc.vector.tensor_scalar_max(out=mx[:], in0=y2[:],
                                                scalar1=0.0)
                    yo = ep.tile([P, 128], F32)
                    nc.vector.scalar_tensor_tensor(
                        out=yo[:], in0=mx[:], scalar=-1.0, in1=e1[:],
                        op0=ALU.add, op1=ALU.add)
                    nc.scalar.dma_start(out=out[b * P:(b + 1) * P, :],
                                        in_=yo[:])

    nc.compile()
    return nc


# ---------------------------------------------------------------------------
# Public entry point.


def run_full(inputs, trace=False, **spmd_kwargs):
    cfg = Cfg()
    in_maps, cA, cB, perm = make_inputs(cfg,
                                        **{k: np.asarray(v) for k, v in
                                           inputs.items()})
    # One SPMD program must serve all 8 cores: use the per-position max count
    # across cores (tight, since each core sorted its blocks by count).
    nA = cA.max(axis=0)
    nB = cB.max(axis=0)
    nc = build_program(cfg, nA, nB)
    res = run_bass_kernel_spmd(nc, in_maps, list(range(cfg.NC)), trace=trace,
                               **spmd_kwargs)
    outs = []
    for c in range(cfg.NC):
        o = np.asarray(res.results[c]["out"]).reshape(cfg.NBLK, P, 128)
        inv = np.empty(cfg.NBLK, dtype=np.int64)
        inv[perm[c]] = np.arange(cfg.NBLK)
        outs.append(o[inv].reshape(cfg.NLOCP, 128)[:cfg.NLOC])
    return np.concatenate(outs, axis=0).astype(np.float32), res


def kernel(x, edge_index, W, att_src, att_dst, bias, skip_W, skip_b):
    out, _ = run_full(dict(x=x, edge_index=edge_index, W=W, att_src=att_src,
                           att_dst=att_dst, bias=bias, skip_W=skip_W,
                           skip_b=skip_b))
    return out
